# revision 24
# baseline (speedup 1.0000x reference)
"""Trainium2 Bass kernel for nn_Complex_Only_46308337385506 (gnn_message_passing).

Math (validated against the jax reference by the prior baseline):
  Per edge, the basis R enters only through two scalars:
      gam = Jz/|J|
      a1  = copysign(sqrt(Jx^2+Jy^2)/|J|, gam+eps)
  With w = gam*Xz - a1*Xx and c8 = a1*Xz + gam*Xx:
      Y0 = Wa@Xx + (Wa-Wc)@(a1*w) + Wb@(gam*Xy)
      Y1 = Wa@Xy - Wb@c8
      Y2 = Wa@Xz + (Wc-Wa)@(gam*w) + Wb@(a1*Xy)
  VN leaky relu (d = Wd@Y over channels, per (f, point)):
      dot = <Y, d>_3 ; dn2 = <d, d>_3
      out = Y - 0.8*min(dot,0)/(dn2+eps) * d
  d is computed directly from the same matmul RHS as Y using host-fused
  weights (Wd@Wk), eliminating the separate Wd stage entirely.

Implementation strategy (cost-model driven):
  - bf16 end to end: halves DMA bytes, 1-cycle/row matmuls+transposes on PE,
    2x DVE modes on packed elementwise ops. rel-err lands ~1e-2 >> margin
    under the 2e-2 gate.
  - Host repacks X/J as [point, comp, edge] partition-major rows so every
    DMA descriptor moves >=512B contiguous and component views are packed.
  - Per super (1024 points = 8 slots x 128 partitions): per-edge scalars in
    point-major layout (batched across 2 supers), PE-transpose gam/a1/X into
    bf16 PSUM ([2 slot-parities x 64 edges] on partitions), products in the
    transposed domain, 16 accumulating bf16 matmuls (Y and fused-d), VN
    stage reading Y/d PSUM banks directly.
  - Custom DVE ops fuse: q=x^2+y^2, n2=q+z^2, a1=copysign, s2=min(dot,0)*rec0^2.
  - ACT table pinned to reciprocal_sqrt_and_small (Rsqrt/Sign/Copy).
"""

import numpy as np
from contextlib import ExitStack

import concourse.bass as bass
import concourse.bacc as bacc
import concourse.tile as tile
from concourse import mybir
from concourse import bass_utils

F32 = mybir.dt.float32
BF16 = mybir.dt.bfloat16
AF = mybir.ActivationFunctionType
ALU = mybir.AluOpType

EPS = 1e-6
NEG = 0.2

B, C, E = 8, 16384, 64
SUPER = 1024           # points per super-iteration
NSUP = C // SUPER      # 16
ROW = 3 * E            # 192 values per point
NW = 11                # weight mats incl. identity

_CUSTOM_OPS = {}


def _register_custom_dve_ops():
    """Register fused DVE ops (module-level, idempotent):
      SQSUM:   out = Src0^2 + Src1^2
      ADDSQ:   out = Src0 + Src1^2
      CPSIGN:  out = Src0 >= c0 ? Src1 : -Src1      (copysign w/ eps shift)
      MINSQM:  out = min(Src0, 0) * Src1^2          (VN-relu gate)
    """
    if _CUSTOM_OPS:
        return _CUSTOM_OPS
    import numpy as _np
    from concourse import dve_ops
    from concourse.dve_spec import (
        Spec, Src0, Src1, C0, C1, Zero, lower, sq, minn, select, _has_src1)
    from concourse.dve_uop import DveOpSpec
    from concourse.dve_table_gen import dve_ver_for

    def make(name, body, ref):
        spec = Spec(body=body, reference=ref)
        opcode = dve_ops._CUSTOM_DVE_ROW_BASE + len(dve_ops.OPS)
        shas = {}
        for ver in ("v3", "v4"):
            try:
                s = DveOpSpec(name=name, opcode=opcode,
                              uops=lower(spec, ver=ver),
                              rd1_en=_has_src1(spec))
                shas[ver] = s.sha(ver)
            except Exception:
                pass
        op = dve_ops.DveOp(name, spec, subdim=False, uops_sha=shas)
        dve_ops.OPS.append(op)
        dve_ops.CUSTOM_DVE_SPECS[name] = spec
        dve_ops._SUB_OPCODE_FOR_NAME[name] = opcode
        assert opcode < 0x20
        return op

    def flat(a):
        return a.astype(_np.float32).reshape(a.shape[0], -1)

    _CUSTOM_OPS["SQSUM"] = make(
        "SQSUM_ANT", sq(Src0) + sq(Src1),
        lambda in0, in1, s0, s1, imm2:
            flat(in0) ** 2 + flat(in1) ** 2)
    _CUSTOM_OPS["ADDSQ"] = make(
        "ADDSQ_ANT", Src0 + sq(Src1),
        lambda in0, in1, s0, s1, imm2:
            flat(in0) + flat(in1) ** 2)
    _CUSTOM_OPS["CPSIGN"] = make(
        "CPSIGN_ANT", select(Src0 >= C0, Src1, Zero - Src1),
        lambda in0, in1, s0, s1, imm2:
            _np.where(flat(in0) >= s0, flat(in1), -flat(in1)))
    _CUSTOM_OPS["MINSQM"] = make(
        "MINSQM_ANT", minn(Src0, Zero) * sq(Src1),
        lambda in0, in1, s0, s1, imm2:
            _np.minimum(flat(in0), 0.0) * flat(in1) ** 2)
    _CUSTOM_OPS["ADDSQB"] = make(
        "ADDSQB_ANT", (Src0 + sq(Src1)) * C0 + C1,
        lambda in0, in1, s0, s1, imm2:
            (flat(in0) + flat(in1) ** 2) * s0 + s1)
    _CUSTOM_OPS["SQSUMB"] = make(
        "SQSUMB_ANT", (sq(Src0) + sq(Src1)) * C0 + C1,
        lambda in0, in1, s0, s1, imm2:
            (flat(in0) ** 2 + flat(in1) ** 2) * s0 + s1)
    return _CUSTOM_OPS


def _pin_act_table_set(arch: str):
    """Steer the ACT table-set chooser so all funcs used here first-match
    sqrt_and_others -> exactly one table load."""
    from concourse import hw_specs
    tables = hw_specs.get_activation_tables(arch)  # cached dict, mutated
    mine = {AF.Sqrt, AF.Sign, AF.Copy, AF.Identity, AF.Square}
    for name, funcs in tables.items():
        if name != "sqrt_and_others":
            funcs -= mine


def _build_nc():
    global OPS
    OPS = _register_custom_dve_ops()
    nc = bacc.Bacc("TRN2", debug=False)
    _pin_act_table_set(nc.m.arch)

    XS = nc.dram_tensor("XS", [NSUP * 64, 16 * ROW], BF16,
                        kind="ExternalInput").ap()
    JS = nc.dram_tensor("JS", [NSUP * 64, 16 * ROW], BF16,
                        kind="ExternalInput").ap()
    WMM = nc.dram_tensor("WMM", [128, NW * 128], BF16,
                         kind="ExternalInput").ap()
    OUT = nc.dram_tensor("OUT", [NSUP * 128, 8 * ROW], BF16,
                         kind="ExternalOutput").ap()

    # DRAM views: row (u2, p) holds supers (2*u2, 2*u2+1); within a row the
    # layout is [uu:2][s:8][i:3][e:64].
    X3 = XS.rearrange("(v p) (uu w) -> v p uu w", p=128, uu=2)
    J3 = JS.rearrange("(v p) (uu w) -> v p uu w", p=128, uu=2)
    O3 = OUT.rearrange("(u p) w -> u p w", p=128)

    with tile.TileContext(nc) as tc, ExitStack() as ctx:
        const = ctx.enter_context(tc.tile_pool(name="const", bufs=1))
        io = ctx.enter_context(tc.tile_pool(name="io", bufs=3))
        sa = ctx.enter_context(tc.tile_pool(name="sa", bufs=3))
        sxp = ctx.enter_context(tc.tile_pool(name="sxp", bufs=4))
        prp = ctx.enter_context(tc.tile_pool(name="prp", bufs=4))
        s3p = ctx.enter_context(tc.tile_pool(name="s3p", bufs=3))
        outp = ctx.enter_context(tc.tile_pool(name="outp", bufs=3))
        psT = ctx.enter_context(tc.tile_pool(name="psT", bufs=1, space="PSUM"))
        psY = ctx.enter_context(tc.tile_pool(name="psY", bufs=1, space="PSUM"))
        psD = ctx.enter_context(tc.tile_pool(name="psD", bufs=1, space="PSUM"))

        # bias constants for ACT (f32 [128,1])
        b_n2 = const.tile([128, 1], F32, tag="b_n2")
        b_q = const.tile([128, 1], F32, tag="b_q")
        b_sg = const.tile([128, 1], F32, tag="b_sg")
        nc.gpsimd.memset(b_n2[:], 1e-12)
        nc.gpsimd.memset(b_q[:], 1e-20)
        nc.gpsimd.memset(b_sg[:], EPS)

        # weights + identity, loaded once: [128, 11, 128]
        wsb = const.tile([128, NW, 128], BF16, tag="wsb")
        nc.sync.dma_start(wsb[:].rearrange("p n m -> p (n m)"), WMM)
        W_A = wsb[:, 0, :]
        W_2 = wsb[:, 1, :]
        W_2n = wsb[:, 2, :]
        W_B = wsb[:, 3, :]
        W_Bn = wsb[:, 4, :]
        W_DA = wsb[:, 5, :]
        W_D2 = wsb[:, 6, :]
        W_D2n = wsb[:, 7, :]
        W_DB = wsb[:, 8, :]
        W_DBn = wsb[:, 9, :]
        IDT = wsb[:, 10, :]

        for u in range(NSUP):
            if u % 2 == 0:
                xs2 = io.tile([128, 2, ROW * 8], BF16, tag="xs")
                js2 = io.tile([128, 2, ROW * 8], BF16, tag="js")
                nc.sync.dma_start(xs2[:], X3[u // 2])
                nc.sync.dma_start(js2[:], J3[u // 2])

                # ---- stage A on the 2-super pair: per-edge scalars --------
                jv = js2[:].rearrange("p uu (s i e) -> p (uu s) i e",
                                      s=8, i=3, e=64)
                q = sa.tile([128, 16, 64], BF16, tag="q")
                n2 = sa.tile([128, 16, 64], BF16, tag="n2")
                nc.vector._custom_dve(OPS["SQSUM"], out=q[:],
                                      in0=jv[:, :, 0, :], in1=jv[:, :, 1, :])
                nc.vector._custom_dve(OPS["ADDSQ"], out=n2[:],
                                      in0=q[:], in1=jv[:, :, 2, :])
                s_ = sa.tile([128, 16, 64], BF16, tag="s_")
                sq_ = sa.tile([128, 16, 64], BF16, tag="sq_")
                nc.scalar.activation(s_[:], n2[:], AF.Sqrt, bias=b_n2[:])
                nc.scalar.activation(sq_[:], q[:], AF.Sqrt, bias=b_q[:])
                t_ = sa.tile([128, 16, 64], BF16, tag="t_")
                with nc.allow_low_precision("bf16 kernel, 2e-2 gate"):
                    nc.vector.reciprocal(t_[:], s_[:])
                rho = sa.tile([128, 16, 64], BF16, tag="rho")
                gam = sa.tile([128, 16, 64], BF16, tag="gam")
                nc.vector.tensor_tensor(rho[:], sq_[:], t_[:], ALU.mult)
                nc.gpsimd.tensor_tensor(gam[:], jv[:, :, 2, :], t_[:],
                                        ALU.mult)
                sgn = sa.tile([128, 16, 64], BF16, tag="sgn")
                nc.scalar.activation(sgn[:], gam[:], AF.Sign, bias=b_sg[:])
                a1 = sa.tile([128, 16, 64], BF16, tag="a1")
                nc.vector.tensor_tensor(a1[:], sgn[:], rho[:], ALU.mult)

            half = u % 2
            xv = xs2[:, half].rearrange("p (s i e) -> p s i e", s=8, i=3,
                                        e=64)
            gv = gam[:, 8 * half:8 * half + 8]     # [128, 8, 64]
            av = a1[:, 8 * half:8 * half + 8]

            # ---- transposes into bf16 PSUM ---------------------------------
            # partitions become (slot-parity h, edge e); free = (k, point p)
            Tga = psT.tile([128, 2, 512], BF16, tag="Tga")
            TX = psT.tile([128, 2, 512], BF16, tag="TX")
            for k in range(4):
                sl = slice(128 * k, 128 * (k + 1))
                nc.tensor.transpose(Tga[:, 0, sl], gv[:, 2 * k:2 * k + 2, :],
                                    IDT)
                nc.tensor.transpose(Tga[:, 1, sl], av[:, 2 * k:2 * k + 2, :],
                                    IDT)
            Tg = Tga[:, 0]
            Ta = Tga[:, 1]

            # X components through a 2-slot PSUM transit, copied to SBUF
            sx = []
            for i in range(3):
                for k in range(4):
                    sl = slice(128 * k, 128 * (k + 1))
                    nc.tensor.transpose(TX[:, i % 2, sl],
                                        xv[:, 2 * k:2 * k + 2, i, :], IDT)
                sxi = sxp.tile([128, 512], BF16, tag=f"sx{i}")
                nc.scalar.activation(sxi[:], TX[:, i % 2], AF.Copy)
                sx.append(sxi)
            sxx, sxy, sxz = sx

            # ---- products in the transposed domain -------------------------
            m1 = prp.tile([128, 512], BF16, tag="m1")
            m2 = prp.tile([128, 512], BF16, tag="m2")
            w = prp.tile([128, 512], BF16, tag="w")
            nc.vector.tensor_tensor(m1[:], Tg, sxz[:], ALU.mult)
            nc.vector.tensor_tensor(m2[:], Ta, sxx[:], ALU.mult)
            nc.vector.tensor_tensor(w[:], m1[:], m2[:], ALU.subtract)
            pw = prp.tile([128, 512], BF16, tag="pw")
            gw = prp.tile([128, 512], BF16, tag="gw")
            gy = prp.tile([128, 512], BF16, tag="gy")
            ay = prp.tile([128, 512], BF16, tag="ay")
            nc.gpsimd.tensor_tensor(pw[:], Ta, w[:], ALU.mult)
            nc.gpsimd.tensor_tensor(gw[:], Tg, w[:], ALU.mult)
            nc.gpsimd.tensor_tensor(gy[:], Tg, sxy[:], ALU.mult)
            nc.gpsimd.tensor_tensor(ay[:], Ta, sxy[:], ALU.mult)
            # c8 = m3 + m4 is folded into the matmuls (extra Bn accumulate)
            m3 = prp.tile([128, 512], BF16, tag="m3")
            m4 = prp.tile([128, 512], BF16, tag="m4")
            nc.gpsimd.tensor_tensor(m3[:], Ta, sxz[:], ALU.mult)
            nc.vector.tensor_tensor(m4[:], Tg, sxx[:], ALU.mult)

            # ---- Y and fused-d matmuls (interleaved so stage-3 starts early)
            Y = [psY.tile([128, 512], F32, tag=f"Y{i}", name=f"Y{i}")
                 for i in range(3)]
            D = [psD.tile([128, 512], F32, tag=f"D{i}", name=f"D{i}")
                 for i in range(3)]
            mm_plan = (
                ((W_A, sxx), (W_2, pw), (W_B, gy)),
                ((W_A, sxy), (W_Bn, m3), (W_Bn, m4)),
                ((W_A, sxz), (W_2n, gw), (W_B, ay)),
            )
            mm_plan_d = (
                ((W_DA, sxx), (W_D2, pw), (W_DB, gy)),
                ((W_DA, sxy), (W_DBn, m3), (W_DBn, m4)),
                ((W_DA, sxz), (W_D2n, gw), (W_DB, ay)),
            )
            for i in range(3):
                for ps, plan in ((D, mm_plan_d[i]), (Y, mm_plan[i])):
                    last = len(plan) - 1
                    for j, (wm, rh) in enumerate(plan):
                        nc.tensor.matmul(ps[i][:], wm, rh[:], start=(j == 0),
                                         stop=(j == last))

            # ---- VN leaky relu --------------------------------------------
            # Early PSUM readers: xd products, Y->SBUF copies, dn2 customs.
            # Late consumers (ot) read the SBUF copies so Y banks free early;
            # D banks free at mi (just after s2).
            xd0 = s3p.tile([128, 512], BF16, tag="xd0")
            xd1 = s3p.tile([128, 512], BF16, tag="xd1")
            xd2 = s3p.tile([128, 512], BF16, tag="xd2")
            nc.gpsimd.tensor_tensor(xd0[:], Y[0][:], D[0][:], ALU.mult)
            nc.gpsimd.tensor_tensor(xd1[:], Y[1][:], D[1][:], ALU.mult)
            nc.gpsimd.tensor_tensor(xd2[:], Y[2][:], D[2][:], ALU.mult)
            ysb = s3p.tile([128, 3, 512], BF16, tag="ysb")
            for i in range(3):
                nc.scalar.activation(ysb[:, i, :], Y[i][:], AF.Copy)
            # D -> SBUF so the late mi reads don't pin the D psum banks
            dsb = s3p.tile([128, 3, 512], BF16, tag="dsb")
            nc.vector.tensor_copy(dsb[:, 0, :], D[0][:])
            nc.gpsimd.tensor_copy(dsb[:, 1, :], D[1][:])
            nc.scalar.activation(dsb[:, 2, :], D[2][:], AF.Copy)
            # dn2s = 1.25*(|d|^2 + eps), assembled as
            #   SQSUMB(D0,D1)*1.25 + 1.25eps  +  Square(sqrt(1.25)*D2)
            dn2 = s3p.tile([128, 512], BF16, tag="dn2")
            nc.vector._custom_dve(OPS["SQSUMB"], out=dn2[:],
                                  in0=D[0][:], in1=D[1][:],
                                  s0=1.25, s1=1.25 * EPS)
            d2s = s3p.tile([128, 512], BF16, tag="d2s")
            nc.scalar.activation(d2s[:], D[2][:], AF.Square,
                                 scale=1.1180340051651001)
            dn2s = s3p.tile([128, 512], BF16, tag="dn2s")
            nc.vector.tensor_tensor(dn2s[:], dn2[:], d2s[:], ALU.add)
            dot = s3p.tile([128, 512], BF16, tag="dot")
            nc.gpsimd.tensor_tensor(dot[:], xd0[:], xd1[:], ALU.add)
            nc.gpsimd.tensor_tensor(dot[:], dot[:], xd2[:], ALU.add)
            # rcp = 0.8/(|d|^2+eps);  s2 = min(dot,0)*rcp
            rcp = s3p.tile([128, 512], BF16, tag="rcp")
            with nc.allow_low_precision("bf16 kernel, 2e-2 gate"):
                nc.vector.reciprocal(rcp[:], dn2s[:])
            s2 = s3p.tile([128, 512], BF16, tag="s2")
            nc.gpsimd.scalar_tensor_tensor(s2[:], dot[:], 0.0, rcp[:],
                                           ALU.min, ALU.mult)

            ot = outp.tile([128, 3, 512], BF16, tag="ot")
            mis = []
            for i in range(3):
                mi = s3p.tile([128, 512], BF16, tag=f"mi{i}", name=f"mi{i}")
                nc.gpsimd.tensor_tensor(mi[:], s2[:], dsb[:, i, :], ALU.mult)
                mis.append(mi)
            for i in range(3):
                nc.vector.tensor_tensor(ot[:, i, :], ysb[:, i, :], mis[i][:],
                                        ALU.subtract)

            nc.sync.dma_start(O3[u], ot[:].rearrange("p i n -> p (i n)"))

    nc.compile()
    return nc


_NC = None


def _get_nc():
    global _NC
    if _NC is None:
        _NC = _build_nc()
    return _NC


def _to_bf16(a):
    import ml_dtypes
    return np.asarray(a, np.float32).astype(ml_dtypes.bfloat16)


def _pack_input(A):
    """[C, E, 3] f32 -> [NSUP*128, 2*ROW] bf16 with row (u2, p) holding
    supers (2u2, 2u2+1); per-row layout [uu][s][i][e]; point
    c = u*1024 + s*128 + p."""
    a = np.asarray(A, np.float32).reshape(NSUP, 8, 128, E, 3)
    a = a.transpose(0, 2, 1, 4, 3)          # [u, p, s, i, e]
    a = a.reshape(NSUP // 2, 2, 128, ROW * 8).transpose(0, 2, 1, 3)
    return np.ascontiguousarray(_to_bf16(a.reshape(NSUP * 64, 16 * ROW)))


def _unpack_output(o):
    """[NSUP*128, ROW] bf16 -> [64, 3, C] f32. Device row (u, q=h*64+f)
    holds [i][k][p] with c = u*1024 + (2k+h)*128 + p."""
    a = np.asarray(o, np.float32).reshape(NSUP, 2, 64, 3, 4, 128)
    a = a.transpose(2, 3, 0, 4, 1, 5)       # [f, i, u, k, h, p]
    return np.ascontiguousarray(a.reshape(64, 3, C))


def _weight_stack(Wa, Wb, Wc, Wd):
    Wa = np.asarray(Wa, np.float32)
    Wb = np.asarray(Wb, np.float32)
    Wc = np.asarray(Wc, np.float32)
    Wd = np.asarray(Wd, np.float32)
    Z = np.zeros((64, 64), np.float32)

    def blk(m):
        return np.block([[m, Z], [Z, m]]).astype(np.float32)

    W2 = Wa - Wc
    mats = [
        blk(Wa.T), blk(W2.T), blk(-W2.T), blk(Wb.T), blk(-Wb.T),
        blk((Wd @ Wa).T), blk((Wd @ W2).T), blk(-(Wd @ W2).T),
        blk((Wd @ Wb).T), blk(-(Wd @ Wb).T),
        np.eye(128, dtype=np.float32),
    ]
    w = np.stack(mats)                       # [11, 128, 128]
    w = w.transpose(1, 0, 2).reshape(128, NW * 128)
    return np.ascontiguousarray(_to_bf16(w))


def run_full(X, J, Wa, Wb, Wc, Wd, trace=False, trace_kwargs=None):
    nc = _get_nc()
    wmm = _weight_stack(Wa, Wb, Wc, Wd)
    in_maps = []
    for b in range(B):
        in_maps.append({
            "XS": _pack_input(X[b]),
            "JS": _pack_input(J[b]),
            "WMM": wmm,
        })
    res = bass_utils.run_bass_kernel_spmd(
        nc, in_maps, core_ids=list(range(B)), trace=trace,
        **(trace_kwargs or {}))
    out = np.stack([_unpack_output(res.results[b]["OUT"]) for b in range(B)])
    return out.astype(np.float32), res


def kernel(X, J, Wa, Wb, Wc, Wd):
    out, _ = run_full(X, J, Wa, Wb, Wc, Wd)
    return out


# revision 25
# speedup vs baseline: 1.1658x; 1.1658x over previous
"""Trainium2 Bass kernel for nn_Complex_Only_46308337385506 (gnn_message_passing).

Math (validated against the jax reference by the prior baseline):
  Per edge, the basis R enters only through two scalars:
      gam = Jz/|J|
      a1  = copysign(sqrt(Jx^2+Jy^2)/|J|, gam+eps)
  With w = gam*Xz - a1*Xx and c8 = a1*Xz + gam*Xx:
      Y0 = Wa@Xx + (Wa-Wc)@(a1*w) + Wb@(gam*Xy)
      Y1 = Wa@Xy - Wb@c8
      Y2 = Wa@Xz + (Wc-Wa)@(gam*w) + Wb@(a1*Xy)
  VN leaky relu (d = Wd@Y over channels, per (f, point)):
      dot = <Y, d>_3 ; dn2 = <d, d>_3
      out = Y - 0.8*min(dot,0)/(dn2+eps) * d
  d is computed directly from the same matmul RHS as Y using host-fused
  weights (Wd@Wk), eliminating the separate Wd stage.  c8 is folded into the
  matmuls as two extra -Wb accumulates (m3, m4).

Implementation strategy (cost-model driven):
  - bf16 end to end: halves DMA bytes, 1-cycle/row matmuls, 2x DVE modes.
  - ALL tensors are loaded PRE-TRANSPOSED via the DMA xbar transpose
    (dma_start_transpose, 16x128 tiles): X and J arrive as
    [128 = (slot-parity h, edge e), points] tiles, so there are no PE
    transposes, no PSUM transit and no PSUM->SBUF copies on the front end.
    Every elementwise op runs on dense packed bf16 SBUF tiles.
  - Per super (1024 points): stage A (per-edge scalars, batched across 2
    supers), 9 products, 16 accumulating bf16 matmuls (Y + fused-d), VN
    stage with early Y/D->SBUF copies so PSUM banks recycle quickly.
  - Custom DVE ops fuse: q=x^2+y^2, n2=q+z^2, dn2=1.25(d0^2+d1^2)+b.
  - ACT table pinned to sqrt_and_others (Sqrt/Sign/Square/Copy).
"""

import numpy as np
from contextlib import ExitStack

import concourse.bass as bass
import concourse.bacc as bacc
import concourse.tile as tile
from concourse import mybir
from concourse import bass_utils

F32 = mybir.dt.float32
BF16 = mybir.dt.bfloat16
AF = mybir.ActivationFunctionType
ALU = mybir.AluOpType

EPS = 1e-6

B, C, E = 8, 16384, 64
SUPER = 1024           # points per super-iteration
NSUP = C // SUPER      # 16
ROW = 3 * E
NW = 10                # weight mats

_CUSTOM_OPS = {}


def _register_custom_dve_ops():
    """Fused DVE ops (module-level, idempotent):
      SQSUM:   out = Src0^2 + Src1^2
      ADDSQ:   out = Src0 + Src1^2
      SQSUMB:  out = (Src0^2 + Src1^2)*c0 + c1
    """
    if _CUSTOM_OPS:
        return _CUSTOM_OPS
    import numpy as _np
    from concourse import dve_ops
    from concourse.dve_spec import (
        Spec, Src0, Src1, C0, C1, lower, sq, _has_src1)
    from concourse.dve_uop import DveOpSpec

    def make(name, body, ref):
        spec = Spec(body=body, reference=ref)
        opcode = dve_ops._CUSTOM_DVE_ROW_BASE + len(dve_ops.OPS)
        shas = {}
        for ver in ("v3", "v4"):
            try:
                s = DveOpSpec(name=name, opcode=opcode,
                              uops=lower(spec, ver=ver),
                              rd1_en=_has_src1(spec))
                shas[ver] = s.sha(ver)
            except Exception:
                pass
        op = dve_ops.DveOp(name, spec, subdim=False, uops_sha=shas)
        dve_ops.OPS.append(op)
        dve_ops.CUSTOM_DVE_SPECS[name] = spec
        dve_ops._SUB_OPCODE_FOR_NAME[name] = opcode
        assert opcode < 0x20
        return op

    def flat(a):
        return a.astype(_np.float32).reshape(a.shape[0], -1)

    _CUSTOM_OPS["SQSUM"] = make(
        "SQSUM_ANT", sq(Src0) + sq(Src1),
        lambda in0, in1, s0, s1, imm2:
            flat(in0) ** 2 + flat(in1) ** 2)
    _CUSTOM_OPS["ADDSQ"] = make(
        "ADDSQ_ANT", Src0 + sq(Src1),
        lambda in0, in1, s0, s1, imm2:
            flat(in0) + flat(in1) ** 2)
    _CUSTOM_OPS["SQSUMB"] = make(
        "SQSUMB_ANT", (sq(Src0) + sq(Src1)) * C0 + C1,
        lambda in0, in1, s0, s1, imm2:
            (flat(in0) ** 2 + flat(in1) ** 2) * s0 + s1)
    return _CUSTOM_OPS


def _pin_act_table_set(arch: str):
    """Steer the ACT table-set chooser so all funcs used here first-match
    sqrt_and_others -> exactly one table load."""
    from concourse import hw_specs
    tables = hw_specs.get_activation_tables(arch)  # cached dict, mutated
    mine = {AF.Sqrt, AF.Sign, AF.Copy, AF.Identity, AF.Square}
    for name, funcs in tables.items():
        if name != "sqrt_and_others":
            funcs -= mine


def _build_nc():
    global OPS
    OPS = _register_custom_dve_ops()
    nc = bacc.Bacc("TRN2", debug=False)
    _pin_act_table_set(nc.m.arch)

    # Transposed-load layout: per (super, comp) a [512, 128] DRAM matrix whose
    # xbar-transpose is the SBUF tile [128=(h,e), 512=(k,p)].
    XS = nc.dram_tensor("XS", [NSUP * 3, 512, 128], BF16,
                        kind="ExternalInput").ap()
    JS = nc.dram_tensor("JS", [NSUP * 3, 512, 128], BF16,
                        kind="ExternalInput").ap()
    WMM = nc.dram_tensor("WMM", [128, NW * 128], BF16,
                         kind="ExternalInput").ap()
    OUT = nc.dram_tensor("OUT", [NSUP * 128, 8 * ROW], BF16,
                         kind="ExternalOutput").ap()

    O3 = OUT.rearrange("(u p) w -> u p w", p=128)

    with tile.TileContext(nc) as tc, ExitStack() as ctx:
        const = ctx.enter_context(tc.tile_pool(name="const", bufs=1))
        io = ctx.enter_context(tc.tile_pool(name="io", bufs=2))
        sa = ctx.enter_context(tc.tile_pool(name="sa", bufs=2))
        prp = ctx.enter_context(tc.tile_pool(name="prp", bufs=2))
        s3p = ctx.enter_context(tc.tile_pool(name="s3p", bufs=2))
        outp = ctx.enter_context(tc.tile_pool(name="outp", bufs=2))
        psY = ctx.enter_context(tc.tile_pool(name="psY", bufs=1, space="PSUM"))
        psD = ctx.enter_context(tc.tile_pool(name="psD", bufs=1, space="PSUM"))

        b_n2 = const.tile([128, 1], F32, tag="b_n2")
        b_q = const.tile([128, 1], F32, tag="b_q")
        b_sg = const.tile([128, 1], F32, tag="b_sg")
        nc.gpsimd.memset(b_n2[:], 1e-12)
        nc.gpsimd.memset(b_q[:], 1e-20)
        nc.gpsimd.memset(b_sg[:], EPS)

        wsb = const.tile([128, NW, 128], BF16, tag="wsb")
        nc.sync.dma_start(wsb[:].rearrange("p n m -> p (n m)"), WMM)
        (W_A, W_2, W_2n, W_B, W_Bn,
         W_DA, W_D2, W_D2n, W_DB, W_DBn) = (wsb[:, i, :] for i in range(NW))

        for u in range(NSUP):
            if u % 2 == 0:
                # transposed loads for 2 supers: [128, 2, 512] per comp
                xT = [io.tile([128, 2, 512], BF16, tag=f"xT{i}",
                              name=f"xT{i}") for i in range(3)]
                jT = [io.tile([128, 2, 512], BF16, tag=f"jT{i}",
                              name=f"jT{i}") for i in range(3)]
                for i in range(3):
                    for h in range(2):
                        nc.sync.dma_start_transpose(
                            xT[i][:, h], XS[(u + h) * 3 + i])
                        nc.sync.dma_start_transpose(
                            jT[i][:, h], JS[(u + h) * 3 + i])

                # ---- stage A on the 2-super pair (transposed layout) ------
                q = sa.tile([128, 2, 512], BF16, tag="q")
                n2 = sa.tile([128, 2, 512], BF16, tag="n2")
                nc.vector._custom_dve(OPS["SQSUM"], out=q[:],
                                      in0=jT[0][:], in1=jT[1][:])
                nc.vector._custom_dve(OPS["ADDSQ"], out=n2[:],
                                      in0=q[:], in1=jT[2][:])
                s_ = sa.tile([128, 2, 512], BF16, tag="s_")
                sq_ = sa.tile([128, 2, 512], BF16, tag="sq_")
                nc.scalar.activation(s_[:], n2[:], AF.Sqrt, bias=b_n2[:])
                nc.scalar.activation(sq_[:], q[:], AF.Sqrt, bias=b_q[:])
                t_ = sa.tile([128, 2, 512], BF16, tag="t_")
                with nc.allow_low_precision("bf16 kernel, 2e-2 gate"):
                    nc.vector.reciprocal(t_[:], s_[:])
                rho = sa.tile([128, 2, 512], BF16, tag="rho")
                gam = sa.tile([128, 2, 512], BF16, tag="gam")
                nc.vector.tensor_tensor(rho[:], sq_[:], t_[:], ALU.mult)
                nc.gpsimd.tensor_tensor(gam[:], jT[2][:], t_[:], ALU.mult)
                sgn = sa.tile([128, 2, 512], BF16, tag="sgn")
                nc.scalar.activation(sgn[:], gam[:], AF.Sign, bias=b_sg[:])
                a1 = sa.tile([128, 2, 512], BF16, tag="a1")
                nc.vector.tensor_tensor(a1[:], sgn[:], rho[:], ALU.mult)

            half = u % 2
            Tg = gam[:, half]
            Ta = a1[:, half]
            tXx = xT[0][:, half]
            tXy = xT[1][:, half]
            tXz = xT[2][:, half]

            # ---- products (all dense SBUF bf16) ----------------------------
            m1 = prp.tile([128, 512], BF16, tag="m1")
            m2 = prp.tile([128, 512], BF16, tag="m2")
            w = prp.tile([128, 512], BF16, tag="w")
            nc.vector.tensor_tensor(m1[:], Tg, tXz, ALU.mult)
            nc.vector.tensor_tensor(m2[:], Ta, tXx, ALU.mult)
            nc.vector.tensor_tensor(w[:], m1[:], m2[:], ALU.subtract)
            pw = prp.tile([128, 512], BF16, tag="pw")
            gw = prp.tile([128, 512], BF16, tag="gw")
            gy = prp.tile([128, 512], BF16, tag="gy")
            ay = prp.tile([128, 512], BF16, tag="ay")
            m3 = prp.tile([128, 512], BF16, tag="m3")
            m4 = prp.tile([128, 512], BF16, tag="m4")
            nc.gpsimd.tensor_tensor(pw[:], Ta, w[:], ALU.mult)
            nc.gpsimd.tensor_tensor(gw[:], Tg, w[:], ALU.mult)
            nc.gpsimd.tensor_tensor(gy[:], Tg, tXy, ALU.mult)
            nc.gpsimd.tensor_tensor(ay[:], Ta, tXy, ALU.mult)
            nc.gpsimd.tensor_tensor(m3[:], Ta, tXz, ALU.mult)
            nc.gpsimd.tensor_tensor(m4[:], Tg, tXx, ALU.mult)

            # ---- Y and fused-d matmuls -------------------------------------
            Y = [psY.tile([128, 512], F32, tag=f"Y{i}", name=f"Y{i}")
                 for i in range(3)]
            D = [psD.tile([128, 512], F32, tag=f"D{i}", name=f"D{i}")
                 for i in range(3)]
            mm_plan = (
                ((W_A, tXx), (W_2, pw[:]), (W_B, gy[:])),
                ((W_A, tXy), (W_Bn, m3[:]), (W_Bn, m4[:])),
                ((W_A, tXz), (W_2n, gw[:]), (W_B, ay[:])),
            )
            mm_plan_d = (
                ((W_DA, tXx), (W_D2, pw[:]), (W_DB, gy[:])),
                ((W_DA, tXy), (W_DBn, m3[:]), (W_DBn, m4[:])),
                ((W_DA, tXz), (W_D2n, gw[:]), (W_DB, ay[:])),
            )
            for i in range(3):
                for ps, plan in ((D, mm_plan_d[i]), (Y, mm_plan[i])):
                    for j, (wm, rh) in enumerate(plan):
                        nc.tensor.matmul(ps[i][:], wm, rh, start=(j == 0),
                                         stop=(j == 2))

            # ---- VN leaky relu --------------------------------------------
            # early PSUM readers; late consumers read SBUF copies
            xd0 = s3p.tile([128, 512], BF16, tag="xd0")
            xd1 = s3p.tile([128, 512], BF16, tag="xd1")
            xd2 = s3p.tile([128, 512], BF16, tag="xd2")
            nc.gpsimd.tensor_tensor(xd0[:], Y[0][:], D[0][:], ALU.mult)
            nc.gpsimd.tensor_tensor(xd1[:], Y[1][:], D[1][:], ALU.mult)
            nc.gpsimd.tensor_tensor(xd2[:], Y[2][:], D[2][:], ALU.mult)
            ysb = s3p.tile([128, 3, 512], BF16, tag="ysb")
            dsb = s3p.tile([128, 3, 512], BF16, tag="dsb")
            for i in range(3):
                nc.scalar.activation(ysb[:, i, :], Y[i][:], AF.Copy)
                nc.scalar.activation(dsb[:, i, :], D[i][:], AF.Copy)
            # dn2s = 1.25*(|d|^2 + eps)
            dn2 = s3p.tile([128, 512], BF16, tag="dn2")
            nc.vector._custom_dve(OPS["SQSUMB"], out=dn2[:],
                                  in0=D[0][:], in1=D[1][:],
                                  s0=1.25, s1=1.25 * EPS)
            d2s = s3p.tile([128, 512], BF16, tag="d2s")
            nc.scalar.activation(d2s[:], D[2][:], AF.Square,
                                 scale=1.1180340051651001)
            dn2s = s3p.tile([128, 512], BF16, tag="dn2s")
            nc.vector.tensor_tensor(dn2s[:], dn2[:], d2s[:], ALU.add)
            dot = s3p.tile([128, 512], BF16, tag="dot")
            nc.gpsimd.tensor_tensor(dot[:], xd0[:], xd1[:], ALU.add)
            nc.gpsimd.tensor_tensor(dot[:], dot[:], xd2[:], ALU.add)
            # rcp = 0.8/(|d|^2+eps);  s2 = min(dot,0)*rcp
            rcp = s3p.tile([128, 512], BF16, tag="rcp")
            with nc.allow_low_precision("bf16 kernel, 2e-2 gate"):
                nc.vector.reciprocal(rcp[:], dn2s[:])
            s2 = s3p.tile([128, 512], BF16, tag="s2")
            nc.gpsimd.scalar_tensor_tensor(s2[:], dot[:], 0.0, rcp[:],
                                           ALU.min, ALU.mult)

            ot = outp.tile([128, 3, 512], BF16, tag="ot")
            for i in range(3):
                mi = s3p.tile([128, 512], BF16, tag=f"mi{i}", name=f"mi{i}")
                nc.gpsimd.tensor_tensor(mi[:], s2[:], dsb[:, i, :], ALU.mult)
                nc.vector.tensor_tensor(ot[:, i, :], ysb[:, i, :], mi[:],
                                        ALU.subtract)

            nc.sync.dma_start(O3[u], ot[:].rearrange("p i n -> p (i n)"))

    nc.compile()
    return nc


_NC = None


def _get_nc():
    global _NC
    if _NC is None:
        _NC = _build_nc()
    return _NC


def _to_bf16(a):
    import ml_dtypes
    return np.asarray(a, np.float32).astype(ml_dtypes.bfloat16)


def _pack_input(A):
    """[C, E, 3] f32 -> [NSUP*3, 512, 128] bf16: per (super u, comp i) the
    matrix M[(k,p), (h,e)] = A[u*1024 + (2k+h)*128 + p, e, i]."""
    a = np.asarray(A, np.float32).reshape(NSUP, 4, 2, 128, E, 3)
    a = a.transpose(0, 5, 1, 3, 2, 4)       # [u, i, k, p, h, e]
    return np.ascontiguousarray(_to_bf16(a.reshape(NSUP * 3, 512, 128)))


def _unpack_output(o):
    """[NSUP*128, 8*ROW] bf16 -> [64, 3, C] f32. Device row (u, q=h*64+f)
    holds [i][k][p] with c = u*1024 + (2k+h)*128 + p."""
    a = np.asarray(o, np.float32).reshape(NSUP, 2, 64, 3, 4, 128)
    a = a.transpose(2, 3, 0, 4, 1, 5)       # [f, i, u, k, h, p]
    return np.ascontiguousarray(a.reshape(64, 3, C))


def _weight_stack(Wa, Wb, Wc, Wd):
    Wa = np.asarray(Wa, np.float32)
    Wb = np.asarray(Wb, np.float32)
    Wc = np.asarray(Wc, np.float32)
    Wd = np.asarray(Wd, np.float32)
    Z = np.zeros((64, 64), np.float32)

    def blk(m):
        return np.block([[m, Z], [Z, m]]).astype(np.float32)

    W2 = Wa - Wc
    mats = [
        blk(Wa.T), blk(W2.T), blk(-W2.T), blk(Wb.T), blk(-Wb.T),
        blk((Wd @ Wa).T), blk((Wd @ W2).T), blk(-(Wd @ W2).T),
        blk((Wd @ Wb).T), blk(-(Wd @ Wb).T),
    ]
    w = np.stack(mats)                       # [10, 128, 128]
    w = w.transpose(1, 0, 2).reshape(128, NW * 128)
    return np.ascontiguousarray(_to_bf16(w))


def run_full(X, J, Wa, Wb, Wc, Wd, trace=False, trace_kwargs=None):
    nc = _get_nc()
    wmm = _weight_stack(Wa, Wb, Wc, Wd)
    in_maps = []
    for b in range(B):
        in_maps.append({
            "XS": _pack_input(X[b]),
            "JS": _pack_input(J[b]),
            "WMM": wmm,
        })
    res = bass_utils.run_bass_kernel_spmd(
        nc, in_maps, core_ids=list(range(B)), trace=trace,
        **(trace_kwargs or {}))
    out = np.stack([_unpack_output(res.results[b]["OUT"]) for b in range(B)])
    return out.astype(np.float32), res


def kernel(X, J, Wa, Wb, Wc, Wd):
    out, _ = run_full(X, J, Wa, Wb, Wc, Wd)
    return out


# revision 26
# speedup vs baseline: 1.2355x; 1.0598x over previous
"""Trainium2 Bass kernel for nn_Complex_Only_46308337385506 (gnn_message_passing).

Math (validated against the jax reference by the prior baseline):
  Per edge, the basis R enters only through two scalars:
      gam = Jz/|J|
      a1  = copysign(sqrt(Jx^2+Jy^2)/|J|, gam+eps)
  With w = gam*Xz - a1*Xx and c8 = a1*Xz + gam*Xx:
      Y0 = Wa@Xx + (Wa-Wc)@(a1*w) + Wb@(gam*Xy)
      Y1 = Wa@Xy - Wb@c8
      Y2 = Wa@Xz + (Wc-Wa)@(gam*w) + Wb@(a1*Xy)
  VN leaky relu (d = Wd@Y over channels, per (f, point)):
      dot = <Y, d>_3 ; dn2 = <d, d>_3
      out = Y - 0.8*min(dot,0)/(dn2+eps) * d
  d is computed directly from the same matmul RHS as Y using host-fused
  weights (Wd@Wk), eliminating the separate Wd stage.  c8 is folded into the
  matmuls as two extra -Wb accumulates (m3, m4).

Implementation strategy (cost-model driven):
  - bf16 end to end: halves DMA bytes, 1-cycle/row matmuls, 2x DVE modes.
  - ALL tensors are loaded PRE-TRANSPOSED via the DMA xbar transpose
    (dma_start_transpose, 16x128 tiles): X and J arrive as
    [128 = (slot-parity h, edge e), points] tiles, so there are no PE
    transposes, no PSUM transit and no PSUM->SBUF copies on the front end.
    Every elementwise op runs on dense packed bf16 SBUF tiles.
  - Per super (1024 points): stage A (per-edge scalars, batched across 2
    supers), 9 products, 16 accumulating bf16 matmuls (Y + fused-d), VN
    stage with early Y/D->SBUF copies so PSUM banks recycle quickly.
  - Custom DVE ops fuse: q=x^2+y^2, n2=q+z^2, dn2=1.25(d0^2+d1^2)+b.
  - ACT table pinned to sqrt_and_others (Sqrt/Sign/Square/Copy).
"""

import numpy as np
from contextlib import ExitStack

import concourse.bass as bass
import concourse.bacc as bacc
import concourse.tile as tile
from concourse import mybir
from concourse import bass_utils

F32 = mybir.dt.float32
BF16 = mybir.dt.bfloat16
AF = mybir.ActivationFunctionType
ALU = mybir.AluOpType

EPS = 1e-6

B, C, E = 8, 16384, 64
SUPER = 1024           # points per super-iteration
NSUP = C // SUPER      # 16
ROW = 3 * E
NW = 10                # weight mats

_CUSTOM_OPS = {}


def _register_custom_dve_ops():
    """Fused DVE ops (module-level, idempotent):
      SQSUM:   out = Src0^2 + Src1^2
      ADDSQ:   out = Src0 + Src1^2
      SQSUMB:  out = (Src0^2 + Src1^2)*c0 + c1
    """
    if _CUSTOM_OPS:
        return _CUSTOM_OPS
    import numpy as _np
    from concourse import dve_ops
    from concourse.dve_spec import (
        Spec, Src0, Src1, C0, C1, lower, sq, _has_src1)
    from concourse.dve_uop import DveOpSpec

    def make(name, body, ref):
        spec = Spec(body=body, reference=ref)
        opcode = dve_ops._CUSTOM_DVE_ROW_BASE + len(dve_ops.OPS)
        shas = {}
        for ver in ("v3", "v4"):
            try:
                s = DveOpSpec(name=name, opcode=opcode,
                              uops=lower(spec, ver=ver),
                              rd1_en=_has_src1(spec))
                shas[ver] = s.sha(ver)
            except Exception:
                pass
        op = dve_ops.DveOp(name, spec, subdim=False, uops_sha=shas)
        dve_ops.OPS.append(op)
        dve_ops.CUSTOM_DVE_SPECS[name] = spec
        dve_ops._SUB_OPCODE_FOR_NAME[name] = opcode
        assert opcode < 0x20
        return op

    def flat(a):
        return a.astype(_np.float32).reshape(a.shape[0], -1)

    _CUSTOM_OPS["SQSUM"] = make(
        "SQSUM_ANT", sq(Src0) + sq(Src1),
        lambda in0, in1, s0, s1, imm2:
            flat(in0) ** 2 + flat(in1) ** 2)
    _CUSTOM_OPS["ADDSQ"] = make(
        "ADDSQ_ANT", Src0 + sq(Src1),
        lambda in0, in1, s0, s1, imm2:
            flat(in0) + flat(in1) ** 2)
    _CUSTOM_OPS["SQSUMB"] = make(
        "SQSUMB_ANT", (sq(Src0) + sq(Src1)) * C0 + C1,
        lambda in0, in1, s0, s1, imm2:
            (flat(in0) ** 2 + flat(in1) ** 2) * s0 + s1)
    return _CUSTOM_OPS


def _pin_act_table_set(arch: str):
    """Steer the ACT table-set chooser so all funcs used here first-match
    sqrt_and_others -> exactly one table load."""
    from concourse import hw_specs
    tables = hw_specs.get_activation_tables(arch)  # cached dict, mutated
    mine = {AF.Sqrt, AF.Sign, AF.Copy, AF.Identity, AF.Square}
    for name, funcs in tables.items():
        if name != "sqrt_and_others":
            funcs -= mine


def _build_nc():
    global OPS
    OPS = _register_custom_dve_ops()
    nc = bacc.Bacc("TRN2", debug=False)
    _pin_act_table_set(nc.m.arch)

    # Transposed-load layout: per (super, comp) a [512, 128] DRAM matrix whose
    # xbar-transpose is the SBUF tile [128=(h,e), 512=(k,p)].
    XS = nc.dram_tensor("XS", [NSUP * 3, 512, 128], BF16,
                        kind="ExternalInput").ap()
    JS = nc.dram_tensor("JS", [NSUP * 3, 512, 128], BF16,
                        kind="ExternalInput").ap()
    WMM = nc.dram_tensor("WMM", [128, NW * 128], BF16,
                         kind="ExternalInput").ap()
    OUT = nc.dram_tensor("OUT", [NSUP * 128, 8 * ROW], BF16,
                         kind="ExternalOutput").ap()

    O3 = OUT.rearrange("(u p) w -> u p w", p=128)

    with tile.TileContext(nc) as tc, ExitStack() as ctx:
        const = ctx.enter_context(tc.tile_pool(name="const", bufs=1))
        io = ctx.enter_context(tc.tile_pool(name="io", bufs=2))
        sa = ctx.enter_context(tc.tile_pool(name="sa", bufs=2))
        prp = ctx.enter_context(tc.tile_pool(name="prp", bufs=2))
        s3p = ctx.enter_context(tc.tile_pool(name="s3p", bufs=2))
        outp = ctx.enter_context(tc.tile_pool(name="outp", bufs=2))
        psY = ctx.enter_context(tc.tile_pool(name="psY", bufs=1, space="PSUM"))
        psD = ctx.enter_context(tc.tile_pool(name="psD", bufs=1, space="PSUM"))

        b_n2 = const.tile([128, 1], F32, tag="b_n2")
        b_q = const.tile([128, 1], F32, tag="b_q")
        b_sg = const.tile([128, 1], F32, tag="b_sg")
        nc.gpsimd.memset(b_n2[:], 1e-12)
        nc.gpsimd.memset(b_q[:], 1e-20)
        nc.gpsimd.memset(b_sg[:], EPS)

        wsb = const.tile([128, NW, 128], BF16, tag="wsb")
        nc.sync.dma_start(wsb[:].rearrange("p n m -> p (n m)"), WMM)
        (W_A, W_2, W_2n, W_B, W_Bn,
         W_DA, W_D2, W_D2n, W_DB, W_DBn) = (wsb[:, i, :] for i in range(NW))

        for u in range(NSUP):
            if u % 2 == 0:
                # transposed loads for 2 supers: [128, 2, 512] per comp
                xT = [io.tile([128, 2, 512], BF16, tag=f"xT{i}",
                              name=f"xT{i}") for i in range(3)]
                jT = [io.tile([128, 2, 512], BF16, tag=f"jT{i}",
                              name=f"jT{i}") for i in range(3)]
                for i in range(3):
                    for h in range(2):
                        nc.sync.dma_start_transpose(
                            xT[i][:, h], XS[(u + h) * 3 + i])
                        nc.sync.dma_start_transpose(
                            jT[i][:, h], JS[(u + h) * 3 + i])

                # ---- stage A on the 2-super pair (transposed layout) ------
                q = sa.tile([128, 2, 512], BF16, tag="q")
                n2 = sa.tile([128, 2, 512], BF16, tag="n2")
                nc.vector._custom_dve(OPS["SQSUM"], out=q[:],
                                      in0=jT[0][:], in1=jT[1][:])
                nc.vector._custom_dve(OPS["ADDSQ"], out=n2[:],
                                      in0=q[:], in1=jT[2][:])
                s_ = sa.tile([128, 2, 512], BF16, tag="s_")
                sq_ = sa.tile([128, 2, 512], BF16, tag="sq_")
                nc.scalar.activation(s_[:], n2[:], AF.Sqrt, bias=b_n2[:])
                nc.scalar.activation(sq_[:], q[:], AF.Sqrt, bias=b_q[:])
                t_ = sa.tile([128, 2, 512], BF16, tag="t_")
                with nc.allow_low_precision("bf16 kernel, 2e-2 gate"):
                    nc.vector.reciprocal(t_[:], s_[:])
                rho = sa.tile([128, 2, 512], BF16, tag="rho")
                gam = sa.tile([128, 2, 512], BF16, tag="gam")
                nc.vector.tensor_tensor(rho[:], sq_[:], t_[:], ALU.mult)
                nc.gpsimd.tensor_tensor(gam[:], jT[2][:], t_[:], ALU.mult)
                sgn = sa.tile([128, 2, 512], BF16, tag="sgn")
                nc.scalar.activation(sgn[:], gam[:], AF.Sign, bias=b_sg[:])
                a1 = sa.tile([128, 2, 512], BF16, tag="a1")
                nc.vector.tensor_tensor(a1[:], sgn[:], rho[:], ALU.mult)

            half = u % 2
            Tg = gam[:, half]
            Ta = a1[:, half]
            tXx = xT[0][:, half]
            tXy = xT[1][:, half]
            tXz = xT[2][:, half]

            # ---- products (all dense SBUF bf16) ----------------------------
            m1 = prp.tile([128, 512], BF16, tag="m1")
            m2 = prp.tile([128, 512], BF16, tag="m2")
            w = prp.tile([128, 512], BF16, tag="w")
            nc.vector.tensor_tensor(m1[:], Tg, tXz, ALU.mult)
            nc.vector.tensor_tensor(m2[:], Ta, tXx, ALU.mult)
            nc.vector.tensor_tensor(w[:], m1[:], m2[:], ALU.subtract)
            pw = prp.tile([128, 512], BF16, tag="pw")
            gw = prp.tile([128, 512], BF16, tag="gw")
            gy = prp.tile([128, 512], BF16, tag="gy")
            ay = prp.tile([128, 512], BF16, tag="ay")
            m3 = prp.tile([128, 512], BF16, tag="m3")
            m4 = prp.tile([128, 512], BF16, tag="m4")
            nc.gpsimd.tensor_tensor(pw[:], Ta, w[:], ALU.mult)
            nc.gpsimd.tensor_tensor(gw[:], Tg, w[:], ALU.mult)
            nc.gpsimd.tensor_tensor(gy[:], Tg, tXy, ALU.mult)
            nc.gpsimd.tensor_tensor(ay[:], Ta, tXy, ALU.mult)
            nc.gpsimd.tensor_tensor(m3[:], Ta, tXz, ALU.mult)
            nc.gpsimd.tensor_tensor(m4[:], Tg, tXx, ALU.mult)

            # ---- Y and fused-d matmuls -------------------------------------
            Y = [psY.tile([128, 512], F32, tag=f"Y{i}", name=f"Y{i}")
                 for i in range(3)]
            D = [psD.tile([128, 512], F32, tag=f"D{i}", name=f"D{i}")
                 for i in range(3)]
            mm_plan = (
                ((W_A, tXx), (W_2, pw[:]), (W_B, gy[:])),
                ((W_A, tXy), (W_Bn, m3[:]), (W_Bn, m4[:])),
                ((W_A, tXz), (W_2n, gw[:]), (W_B, ay[:])),
            )
            mm_plan_d = (
                ((W_DA, tXx), (W_D2, pw[:]), (W_DB, gy[:])),
                ((W_DA, tXy), (W_DBn, m3[:]), (W_DBn, m4[:])),
                ((W_DA, tXz), (W_D2n, gw[:]), (W_DB, ay[:])),
            )
            for i in range(3):
                for ps, plan in ((D, mm_plan_d[i]), (Y, mm_plan[i])):
                    for j, (wm, rh) in enumerate(plan):
                        nc.tensor.matmul(ps[i][:], wm, rh, start=(j == 0),
                                         stop=(j == 2))

            # ---- VN leaky relu --------------------------------------------
            # early PSUM readers; late consumers read SBUF copies
            xd0 = s3p.tile([128, 512], BF16, tag="xd0")
            xd1 = s3p.tile([128, 512], BF16, tag="xd1")
            xd2 = s3p.tile([128, 512], BF16, tag="xd2")
            nc.gpsimd.tensor_tensor(xd0[:], Y[0][:], D[0][:], ALU.mult)
            nc.gpsimd.tensor_tensor(xd1[:], Y[1][:], D[1][:], ALU.mult)
            nc.gpsimd.tensor_tensor(xd2[:], Y[2][:], D[2][:], ALU.mult)
            ysb = s3p.tile([128, 3, 512], BF16, tag="ysb")
            dsb = s3p.tile([128, 3, 512], BF16, tag="dsb")
            for i in range(3):
                nc.scalar.activation(ysb[:, i, :], Y[i][:], AF.Copy)
                nc.scalar.activation(dsb[:, i, :], D[i][:], AF.Copy)
            # dn2s = 1.25*(|d|^2 + eps)
            dn2 = s3p.tile([128, 512], BF16, tag="dn2")
            nc.vector._custom_dve(OPS["SQSUMB"], out=dn2[:],
                                  in0=D[0][:], in1=D[1][:],
                                  s0=1.25, s1=1.25 * EPS)
            d2s = s3p.tile([128, 512], BF16, tag="d2s")
            nc.scalar.activation(d2s[:], D[2][:], AF.Square,
                                 scale=1.1180340051651001)
            dn2s = s3p.tile([128, 512], BF16, tag="dn2s")
            nc.vector.tensor_tensor(dn2s[:], dn2[:], d2s[:], ALU.add)
            dot = s3p.tile([128, 512], BF16, tag="dot")
            nc.vector.tensor_tensor(dot[:], xd0[:], xd1[:], ALU.add)
            nc.gpsimd.tensor_tensor(dot[:], dot[:], xd2[:], ALU.add)
            # rcp = 0.8/(|d|^2+eps);  s2 = min(dot,0)*rcp
            rcp = s3p.tile([128, 512], BF16, tag="rcp")
            with nc.allow_low_precision("bf16 kernel, 2e-2 gate"):
                nc.vector.reciprocal(rcp[:], dn2s[:])
            s2 = s3p.tile([128, 512], BF16, tag="s2")
            nc.gpsimd.scalar_tensor_tensor(s2[:], dot[:], 0.0, rcp[:],
                                           ALU.min, ALU.mult)

            ot = outp.tile([128, 3, 512], BF16, tag="ot")
            for i in range(3):
                mi = s3p.tile([128, 512], BF16, tag=f"mi{i}", name=f"mi{i}")
                nc.gpsimd.tensor_tensor(mi[:], s2[:], dsb[:, i, :], ALU.mult)
                nc.vector.tensor_tensor(ot[:, i, :], ysb[:, i, :], mi[:],
                                        ALU.subtract)

            nc.sync.dma_start(O3[u], ot[:].rearrange("p i n -> p (i n)"))

    nc.compile()
    return nc


_NC = None


def _get_nc():
    global _NC
    if _NC is None:
        _NC = _build_nc()
    return _NC


def _to_bf16(a):
    import ml_dtypes
    return np.asarray(a, np.float32).astype(ml_dtypes.bfloat16)


def _pack_input(A):
    """[C, E, 3] f32 -> [NSUP*3, 512, 128] bf16: per (super u, comp i) the
    matrix M[(k,p), (h,e)] = A[u*1024 + (2k+h)*128 + p, e, i]."""
    a = np.asarray(A, np.float32).reshape(NSUP, 4, 2, 128, E, 3)
    a = a.transpose(0, 5, 1, 3, 2, 4)       # [u, i, k, p, h, e]
    return np.ascontiguousarray(_to_bf16(a.reshape(NSUP * 3, 512, 128)))


def _unpack_output(o):
    """[NSUP*128, 8*ROW] bf16 -> [64, 3, C] f32. Device row (u, q=h*64+f)
    holds [i][k][p] with c = u*1024 + (2k+h)*128 + p."""
    a = np.asarray(o, np.float32).reshape(NSUP, 2, 64, 3, 4, 128)
    a = a.transpose(2, 3, 0, 4, 1, 5)       # [f, i, u, k, h, p]
    return np.ascontiguousarray(a.reshape(64, 3, C))


def _weight_stack(Wa, Wb, Wc, Wd):
    Wa = np.asarray(Wa, np.float32)
    Wb = np.asarray(Wb, np.float32)
    Wc = np.asarray(Wc, np.float32)
    Wd = np.asarray(Wd, np.float32)
    Z = np.zeros((64, 64), np.float32)

    def blk(m):
        return np.block([[m, Z], [Z, m]]).astype(np.float32)

    W2 = Wa - Wc
    mats = [
        blk(Wa.T), blk(W2.T), blk(-W2.T), blk(Wb.T), blk(-Wb.T),
        blk((Wd @ Wa).T), blk((Wd @ W2).T), blk(-(Wd @ W2).T),
        blk((Wd @ Wb).T), blk(-(Wd @ Wb).T),
    ]
    w = np.stack(mats)                       # [10, 128, 128]
    w = w.transpose(1, 0, 2).reshape(128, NW * 128)
    return np.ascontiguousarray(_to_bf16(w))


def run_full(X, J, Wa, Wb, Wc, Wd, trace=False, trace_kwargs=None):
    nc = _get_nc()
    wmm = _weight_stack(Wa, Wb, Wc, Wd)
    in_maps = []
    for b in range(B):
        in_maps.append({
            "XS": _pack_input(X[b]),
            "JS": _pack_input(J[b]),
            "WMM": wmm,
        })
    res = bass_utils.run_bass_kernel_spmd(
        nc, in_maps, core_ids=list(range(B)), trace=trace,
        **(trace_kwargs or {}))
    out = np.stack([_unpack_output(res.results[b]["OUT"]) for b in range(B)])
    return out.astype(np.float32), res


def kernel(X, J, Wa, Wb, Wc, Wd):
    out, _ = run_full(X, J, Wa, Wb, Wc, Wd)
    return out
